# revision 1
# baseline (speedup 1.0000x reference)
import numpy as np
import jax
import jax.numpy as jnp
from jax.sharding import Mesh, PartitionSpec as P
from jax.experimental.shard_map import shard_map
from functools import partial

# Problem constants (hardcoded per spec)
B, L, D, N = 4, 4096, 1024, 512
LN_EPS = 1e-5
CH = 128          # chunk length
NC = L // CH      # 32 chunks


_IDX = np.arange(CH)[:, None] - np.arange(CH)[None, :]
_MASK = (_IDX >= 0)[:, :, None]
_IDXC = np.clip(_IDX, 0, CH - 1)


def _dss_shard(x, A1r, A1i, EPr, EPi, E2r, E2i, Ctr, Cti, Kloc, Dh, g, bta, didx):
    # build per-d triangular toeplitz on device from Kloc [CH, Dh]
    Ttoe = jnp.where(_MASK, Kloc[_IDXC, :], 0.0)
    # x: [1, L, D] full-channel batch shard; everything else local d-half (Dh=512)
    xb = x[0]                                    # [L, D]
    mu = jnp.mean(xb, axis=-1, keepdims=True)
    var = jnp.var(xb, axis=-1, keepdims=True)
    un = (xb - mu) * jax.lax.rsqrt(var + LN_EPS) * g + bta   # [L, D]
    h = jax.lax.axis_index('h')
    u = jax.lax.dynamic_slice(un, (0, h * (D // 2)), (L, D // 2))  # [L, 512]

    uc = u.reshape(NC, CH, D // 2)               # [c, s, d]
    # local (per-chunk) states: Sloc[c,n,d] = sum_s A1[s,n] * u[c,s,d]
    Slr = jnp.einsum('sn,csd->cnd', A1r, uc)
    Sli = jnp.einsum('sn,csd->cnd', A1i, uc)

    # scan over chunks: S[c] = EP*S[c-1] + Sloc[c-1]  (complex diag per n)
    def step(carry, sl):
        sr, si = carry
        slr, sli = sl
        nsr = EPr[:, None] * sr - EPi[:, None] * si + slr
        nsi = EPr[:, None] * si + EPi[:, None] * sr + sli
        return (nsr, nsi), (sr, si)
    z = jnp.zeros((N, D // 2), jnp.float32)
    try:
        z = jax.lax.pcast(z, ('b', 'h'), to='varying')
    except AttributeError:
        z = jax.lax.pvary(z, ('b', 'h'))
    _, (Spr, Spi) = jax.lax.scan(step, (z, z), (Slr, Sli))
    # Spr[c] = state BEFORE chunk c? scan emits carry before update, with inputs
    # Sloc[c]: emitted carry at step c is S after chunks < c... check: at step c,
    # emit (sr,si) = state from chunks [0..c-1] then update with Sloc[c]. Correct.

    # W = Ct (conj layout [n,d]) hadamard S
    Wr = Ctr * Spr - Cti * Spi
    Wi = Ctr * Spi + Cti * Spr

    # inter-chunk output: y_int[c,t,d] = Re sum_n E2[t,n] W[c,n,d]
    y_int = jnp.einsum('tn,cnd->ctd', E2r, Wr) - jnp.einsum('tn,cnd->ctd', E2i, Wi)

    # intra-chunk causal: y_intra[c,t,d] = sum_{s<=t} Ttoe[t,s,d] u[c,s,d]
    y_intra = jnp.einsum('tsd,csd->ctd', Ttoe, uc)

    y = (y_int + y_intra).reshape(L, D // 2) + u * Dh[None, :]
    return y[None]                               # [1, L, 512]


def kernel(x, Lambda_real, Lambda_imag, C_real, C_imag, param_D, ln_gamma, ln_beta):
    x = np.asarray(x, np.float32)
    # ---- host precompute in float64 ----
    Lr = -np.exp(np.asarray(Lambda_real, np.float64))
    Li = np.exp(np.asarray(Lambda_imag, np.float64))
    lam = Lr + 1j * Li                                    # [N]
    Cc = (np.asarray(C_real, np.float64) + 1j * np.asarray(C_imag, np.float64))
    Ct = Cc * (np.exp(lam) - 1.0) / lam                   # [D, N]

    s = np.arange(CH)
    A1 = np.exp(lam[None, :] * (CH - 1 - s)[:, None])     # [s, n] e^{lam*(CH-1-s)}
    EP = np.exp(lam * CH)                                 # [n]
    t = np.arange(CH)
    E2 = np.exp(lam[None, :] * (t + 1)[:, None])          # [t, n]
    # intra toeplitz per d-half later; K_loc[tau, d] = Re sum_n Ct[d,n] e^{lam tau}
    tau = np.arange(CH)
    Etau = np.exp(lam[None, :] * tau[:, None])            # [tau, n]
    Kloc = np.real(Etau @ Ct.T)                           # [tau, D]

    f32 = lambda a: np.ascontiguousarray(np.real(a), np.float32)
    A1r, A1i = f32(A1), np.ascontiguousarray(np.imag(A1), np.float32)
    EPr, EPi = f32(EP), np.ascontiguousarray(np.imag(EP), np.float32)
    E2r, E2i = f32(E2), np.ascontiguousarray(np.imag(E2), np.float32)
    # Ct in [n, d] layout per half
    CtT = Ct.T                                            # [N, D]
    Ctr = np.ascontiguousarray(np.real(CtT), np.float32)
    Cti = np.ascontiguousarray(np.imag(CtT), np.float32)
    KlocT = np.ascontiguousarray(Kloc, np.float32)        # [CH, D]
    Dv = np.asarray(param_D, np.float32)
    g = np.asarray(ln_gamma, np.float32)
    bta = np.asarray(ln_beta, np.float32)

    mesh, fn, specs = _get_fn()
    didx = np.zeros((), np.int32)
    args = (x, A1r, A1i, EPr, EPi, E2r, E2i, Ctr, Cti, KlocT, Dv, g, bta, didx)
    from jax.sharding import NamedSharding
    dargs = [jax.device_put(a, NamedSharding(mesh, sp)) for a, sp in zip(args, specs)]
    y = fn(*dargs)
    return np.asarray(jax.device_get(y), np.float32)


_CACHE = {}


def _get_fn():
    if 'fn' not in _CACHE:
        devs = np.array(jax.devices()[:8]).reshape(4, 2)
        mesh = Mesh(devs, ('b', 'h'))
        specs = (P('b', None, None), P(), P(), P(), P(), P(), P(),
                 P(None, 'h'), P(None, 'h'), P(None, 'h'), P('h'), P(), P(), P())
        fn = jax.jit(shard_map(_dss_shard, mesh=mesh, in_specs=specs,
                               out_specs=P('b', None, 'h')))
        _CACHE['fn'] = (mesh, fn, specs)
    return _CACHE['fn']



# revision 2
# speedup vs baseline: 2.3845x; 2.3845x over previous
import numpy as np
import jax
import jax.numpy as jnp
from jax.sharding import Mesh, NamedSharding, PartitionSpec as P
from jax.experimental.shard_map import shard_map

# Problem constants (hardcoded per spec)
B, L, D, N = 4, 4096, 1024, 512
LN_EPS = 1e-5
CH = 128          # chunk length
NC = L // CH      # 32 chunks
DH = D // 2       # channels per device (d sharded 2-way)

_IDX = np.arange(CH)[:, None] - np.arange(CH)[None, :]
_MASK = (_IDX >= 0)[:, :, None]
_IDXC = np.clip(_IDX, 0, CH - 1)

_BF16 = jnp.bfloat16


def _dss_shard(x, A1r, A1i, EPr, EPi, E2r, E2i, Ctr, Cti, Kloc, Dh, g, bta):
    # x: [1, L, DH] bf16 — this device's batch element and channel half.
    xb = x[0].astype(jnp.float32)                 # [L, DH]
    # LayerNorm over full D: partial sums + psum over the channel axis pair
    s1 = jnp.sum(xb, axis=-1)                     # [L]
    s2 = jnp.sum(xb * xb, axis=-1)                # [L]
    stats = jnp.stack([s1, s2], axis=0)           # [2, L]
    stats = jax.lax.psum(stats, 'h')
    mu = stats[0] / D
    var = stats[1] / D - mu * mu
    rstd = jax.lax.rsqrt(var + LN_EPS)            # [L]
    u = (xb - mu[:, None]) * rstd[:, None] * g + bta   # [L, DH] f32
    ub = u.astype(_BF16)

    uc = ub.reshape(NC, CH, DH)                   # [c, s, d] bf16
    # local (per-chunk) states: Sloc[c,n,d] = sum_s A1[s,n] * u[c,s,d]
    Slr = jnp.einsum('sn,csd->cnd', A1r, uc, preferred_element_type=jnp.float32)
    Sli = jnp.einsum('sn,csd->cnd', A1i, uc, preferred_element_type=jnp.float32)

    # scan over chunks: emitted state at step c covers chunks < c
    def step(carry, sl):
        sr, si = carry
        slr, sli = sl
        nsr = EPr[:, None] * sr - EPi[:, None] * si + slr
        nsi = EPr[:, None] * si + EPi[:, None] * sr + sli
        return (nsr, nsi), (sr, si)
    z = jnp.zeros((N, DH), jnp.float32)
    try:
        z = jax.lax.pvary(z, ('b', 'h'))
    except AttributeError:
        pass
    _, (Spr, Spi) = jax.lax.scan(step, (z, z), (Slr, Sli))

    # W = Ct (conj layout [n,d]) hadamard S
    Wr = (Ctr * Spr - Cti * Spi).astype(_BF16)
    Wi = (Ctr * Spi + Cti * Spr).astype(_BF16)

    # inter-chunk output: y_int[c,t,d] = Re sum_n E2[t,n] W[c,n,d]
    y_int = (jnp.einsum('tn,cnd->ctd', E2r, Wr, preferred_element_type=jnp.float32)
             - jnp.einsum('tn,cnd->ctd', E2i, Wi, preferred_element_type=jnp.float32))

    # intra-chunk causal: y_intra[c,t,d] = sum_{s<=t} Ttoe[t,s,d] u[c,s,d]
    Ttoe = jnp.where(_MASK, Kloc[_IDXC, :], 0)    # [t, s, d] bf16
    y_intra = jnp.einsum('tsd,csd->ctd', Ttoe, uc, preferred_element_type=jnp.float32)

    y = (y_int + y_intra).reshape(L, DH) + u * Dh[None, :]
    return y.astype(_BF16)[None]                  # [1, L, DH] bf16


def _get_fn():
    if 'fn' not in _CACHE:
        devs = np.array(jax.devices()[:8]).reshape(4, 2)
        mesh = Mesh(devs, ('b', 'h'))
        rep, h = P(), P(None, 'h')
        specs = (P('b', None, 'h'), rep, rep, rep, rep, rep, rep,
                 h, h, h, P('h'), P('h'), P('h'))
        fn = jax.jit(shard_map(_dss_shard, mesh=mesh, in_specs=specs,
                               out_specs=P('b', None, 'h'), check_rep=False))
        _CACHE['fn'] = (mesh, fn, specs)
    return _CACHE['fn']


_CACHE = {}


def kernel(x, Lambda_real, Lambda_imag, C_real, C_imag, param_D, ln_gamma, ln_beta):
    import ml_dtypes
    bf16 = ml_dtypes.bfloat16

    # ---- host precompute in float64 ----
    Lr = -np.exp(np.asarray(Lambda_real, np.float64))
    Li = np.exp(np.asarray(Lambda_imag, np.float64))
    lam = Lr + 1j * Li                                    # [N]
    Cc = (np.asarray(C_real, np.float64) + 1j * np.asarray(C_imag, np.float64))
    Ct = Cc * (np.exp(lam) - 1.0) / lam                   # [D, N]

    s = np.arange(CH)
    A1 = np.exp(lam[None, :] * (CH - 1 - s)[:, None])     # [s, n]
    EP = np.exp(lam * CH)                                 # [n]
    E2 = np.exp(lam[None, :] * (s + 1)[:, None])          # [t, n]
    Etau = np.exp(lam[None, :] * s[:, None])              # [tau, n]
    Kloc = np.real(Etau @ Ct.T)                           # [tau, D]

    fr = lambda a, dt=np.float32: np.ascontiguousarray(np.real(a), dt)
    fi = lambda a, dt=np.float32: np.ascontiguousarray(np.imag(a), dt)
    A1r, A1i = fr(A1, bf16), fi(A1, bf16)
    EPr, EPi = fr(EP), fi(EP)
    E2r, E2i = fr(E2, bf16), fi(E2, bf16)
    CtT = Ct.T                                            # [N, D]
    Ctr, Cti = fr(CtT), fi(CtT)
    KlocT = np.ascontiguousarray(Kloc, bf16)              # [CH, D]
    Dv = np.asarray(param_D, np.float32)
    g = np.asarray(ln_gamma, np.float32)
    bta = np.asarray(ln_beta, np.float32)

    xb = np.asarray(x, np.float32).astype(bf16)           # 32MB over the wire

    mesh, fn, specs = _get_fn()
    args = (xb, A1r, A1i, EPr, EPi, E2r, E2i, Ctr, Cti, KlocT, Dv, g, bta)
    dargs = [jax.device_put(a, NamedSharding(mesh, sp)) for a, sp in zip(args, specs)]
    y = fn(*dargs)
    return np.asarray(jax.device_get(y)).astype(np.float32)


# revision 3
# speedup vs baseline: 3.0740x; 1.2892x over previous
import numpy as np
import jax
import jax.numpy as jnp

# Problem constants (hardcoded per spec)
B, L, D, N = 4, 4096, 1024, 512
LN_EPS = 1e-5
CH = 128          # chunk length
NC = L // CH      # 32 chunks
NDEV = 8
DS = D // NDEV    # channels per device

_IDX = np.arange(CH)[:, None] - np.arange(CH)[None, :]
_MASK = (_IDX >= 0)[:, :, None]
_IDXC = np.clip(_IDX, 0, CH - 1)

_BF16 = jnp.bfloat16
_F32 = jnp.float32


def _dss_dev(q, p, m, Tr, Ti, EPr, EPi, Ctr, Cti, Kl, g, bta, Dv):
    # q: [B,L,DS] int8   p,m: [B,L] f16 (rowscale*rstd, mu*rstd)
    # Tr/Ti: [CH+1,N] bf16 (exp(lam*k))   EPr/i: [N] f32   Ctr/i: [N,DS] f16
    # Kl: [CH,DS] f16    g,bta,Dv: [DS] f32
    u = q.astype(_F32) * p.astype(_F32)[..., None] - m.astype(_F32)[..., None]
    u = u * g + bta                               # [B,L,DS] f32
    ub = u.astype(_BF16)
    uc = ub.reshape(B, NC, CH, DS)

    A1r = jnp.flip(Tr[:CH], 0)                    # [s,n]: exp(lam*(CH-1-s))
    A1i = jnp.flip(Ti[:CH], 0)
    E2r = Tr[1:CH + 1]                            # [t,n]: exp(lam*(t+1))
    E2i = Ti[1:CH + 1]

    # local chunk states: Sloc[c,n,b,d] = sum_s A1[s,n] u[b,c,s,d]
    Slr = jnp.einsum('sn,bcsd->cnbd', A1r, uc, preferred_element_type=_F32)
    Sli = jnp.einsum('sn,bcsd->cnbd', A1i, uc, preferred_element_type=_F32)

    # scan over chunks: emitted state at step c covers chunks < c
    def step(carry, sl):
        sr, si = carry
        slr, sli = sl
        nsr = EPr[:, None, None] * sr - EPi[:, None, None] * si + slr
        nsi = EPr[:, None, None] * si + EPi[:, None, None] * sr + sli
        return (nsr, nsi), (sr, si)
    z = jnp.zeros((N, B, DS), _F32)
    _, (Spr, Spi) = jax.lax.scan(step, (z, z), (Slr, Sli))   # [NC,N,B,DS]

    Cr = Ctr.astype(_F32)[None, :, None, :]
    Ci = Cti.astype(_F32)[None, :, None, :]
    Wr = (Cr * Spr - Ci * Spi).astype(_BF16)
    Wi = (Cr * Spi + Ci * Spr).astype(_BF16)

    y_int = (jnp.einsum('tn,cnbd->bctd', E2r, Wr, preferred_element_type=_F32)
             - jnp.einsum('tn,cnbd->bctd', E2i, Wi, preferred_element_type=_F32))

    Ttoe = jnp.where(_MASK, Kl.astype(_BF16)[_IDXC, :], 0)   # [t,s,d]
    y_intra = jnp.einsum('tsd,bcsd->bctd', Ttoe, uc, preferred_element_type=_F32)

    y = (y_int + y_intra).reshape(B, L, DS) + u * Dv
    return y.astype(_BF16)


_CACHE = {}


def _get_fn():
    if 'fn' not in _CACHE:
        _CACHE['fn'] = jax.jit(_dss_dev)
    return _CACHE['fn']


def kernel(x, Lambda_real, Lambda_imag, C_real, C_imag, param_D, ln_gamma, ln_beta):
    f16 = np.float16
    import ml_dtypes
    bf16 = ml_dtypes.bfloat16

    x32 = np.asarray(x, np.float32)

    # ---- LayerNorm stats + int8 row quantization on host ----
    s1 = x32.sum(-1)
    s2 = np.einsum('bld,bld->bl', x32, x32, optimize=True)
    mu = s1 / D
    var = s2 / D - mu * mu
    rstd = 1.0 / np.sqrt(var + LN_EPS)            # [B,L]
    rm = np.maximum(x32.max(-1), -x32.min(-1))    # [B,L] abs-max per row
    rm = np.maximum(rm, 1e-30)
    inv_scale = 127.0 / rm
    q = np.rint(x32 * inv_scale[..., None]).astype(np.int8)   # [B,L,D]
    p16 = ((rm / 127.0) * rstd).astype(f16)
    m16 = (mu * rstd).astype(f16)

    # ---- kernel tables (float64 host precompute) ----
    lam = -np.exp(np.asarray(Lambda_real, np.float64)) \
        + 1j * np.exp(np.asarray(Lambda_imag, np.float64))     # [N]
    Cc = np.asarray(C_real, np.float64) + 1j * np.asarray(C_imag, np.float64)
    Ct = (Cc * (np.exp(lam) - 1.0) / lam).T                    # [N,D]

    k = np.arange(CH + 1)
    T = np.exp(lam[None, :] * k[:, None])                      # [CH+1,N]
    Tr = np.ascontiguousarray(np.real(T), bf16)
    Ti = np.ascontiguousarray(np.imag(T), bf16)
    EP = T[CH]
    EPr = np.real(EP).astype(np.float32)
    EPi = np.imag(EP).astype(np.float32)
    Kloc = np.real(T[:CH] @ Ct)                                # [CH,D]

    Dv = np.asarray(param_D, np.float32)
    g = np.asarray(ln_gamma, np.float32)
    bta = np.asarray(ln_beta, np.float32)

    fn = _get_fn()
    devs = jax.devices()[:NDEV]

    # ---- stage per-device args, ship in chain order, dispatch async ----
    puts, shards = [], []
    for i, dev in enumerate(devs):
        sl = slice(i * DS, (i + 1) * DS)
        args = (np.ascontiguousarray(q[:, :, sl]), p16, m16, Tr, Ti, EPr, EPi,
                np.ascontiguousarray(np.real(Ct[:, sl]), f16),
                np.ascontiguousarray(np.imag(Ct[:, sl]), f16),
                np.ascontiguousarray(Kloc[:, sl], f16),
                np.ascontiguousarray(g[sl]), np.ascontiguousarray(bta[sl]),
                np.ascontiguousarray(Dv[sl]))
        puts.extend(args)
        shards.extend([dev] * len(args))
    nargs = 13
    dbufs = jax.device_put(puts, shards)
    outs = [fn(*dbufs[i * nargs:(i + 1) * nargs]) for i in range(NDEV)]
    ys = jax.device_get(outs)                     # [B,L,DS] bf16 each

    return np.concatenate(ys, axis=-1).astype(np.float32)
